# revision 20
# baseline (speedup 1.0000x reference)
"""MoE gate kernel for Trainium2 (8 NeuronCores, SPMD data-parallel over tokens).

reference:
    scores = sigmoid(x @ W.T)            # [T, E] fp32
    biased = scores + bias
    inds   = top_k(-biased, 8).indices   # 8 smallest biased, ascending biased
    sel    = scores[inds] / sum * 2.5

Numerics (validated host-side on the exact seeded inputs):
    logits = xh@wh  +  ([e4(xl*2^12); e4(xh)] @ [[e4(wh)]; [e4(wl*2^12)]]) / 2^12
  where xh/wh are the fp16 hi parts and xl/wl the fp32 residuals. The single
  fp8 matmul over the concatenated 8192-deep contraction captures both first-
  order correction terms; combined rel-err vs the fp32 reference ~7e-3.

Device strategy (per core, 2048 tokens):
  - main term: x-stationary fp16 matmuls, PSUM comes out [tokens, experts]
    (32 h-slices x 16 token tiles, N=256).
  - residual: w-stationary fp8 DoubleRow matmuls (256-deep contraction per
    instruction), output [experts, tokens] in PSUM. Two 1024-token passes so
    residual PSUM (4 banks) + open accs (4 banks) fit in 8 banks.
  - fold-in: residual PSUM -> ACT copy to SBUF fp16, then one tiny fp16
    matmul per (tile, expert-half) against diag(2^-12): transposes,
    descales and accumulates into the main acc bank in one PE op (the
    dot products have a single nonzero -> exact).
  - ACT sigmoid (fp32 for the top-8 compare path + fp16 copy for gather),
    DVE max/max_index for top-8, fp16 iota-equality gather of the selected
    original scores, reciprocal-normalize, scale by 2.5.
  - DMA: big transfers only (1-4 MiB), wth+xh on the sync HWDGE ring,
    w8+x8+consts on the scalar ring, deep buffering to avoid ring-head
    blocking.
  - Output: one [128, NT*16] u32 buffer; per token 8 idx words + 8 fp32-bit
    sel words; host un-permutes.
"""

import sys

sys.path.insert(0, "/opt/trn_rl_repo")

import numpy as np
import ml_dtypes

import concourse.bacc as bacc
import concourse.mybir as mybir
import concourse.tile as tile
from concourse import bass_utils

T, H, E, K = 16384, 4096, 256, 8
N_CORES = 8
TS = T // N_CORES          # tokens per core
TCHUNK = 128               # tokens per PE tile (PSUM partition dim)
NT = TS // TCHUNK          # token tiles per core
F = H // 128               # h-slices per partition block
S2 = H * 2                 # fp8 concat contraction depth (8192)
NS = S2 // 256             # DoubleRow slices (32)
TP = TS // 2               # tokens per residual pass (1024)
ROUTED_SCALING = 2.5
RSHIFT = 12                # residual scale = 2^12

f32 = mybir.dt.float32
f16 = mybir.dt.float16
f8 = mybir.dt.float8e4
u32 = mybir.dt.uint32
Alu = mybir.AluOpType
Act = mybir.ActivationFunctionType
DR = mybir.MatmulPerfMode.DoubleRow

E4NP = ml_dtypes.float8_e4m3


def build_nc(nt=NT):
    """Build the SPMD Bass program for one core handling nt*TCHUNK tokens."""
    nc = bacc.Bacc("TRN2", target_bir_lowering=False, debug=False,
                   num_devices=N_CORES)

    # x pre-tiled on host: [it, p, f*TCHUNK + t] = x[it*TCHUNK + t, 32p + f]
    xth_d = nc.dram_tensor("xth", [nt, 128, F * TCHUNK], f16,
                           kind="ExternalInput")
    # fp8 concat residual, moving operand: [pass, k, s, j, t]
    #   h' = s*256 + j*128 + k in [0, 8192); tok = pass*TP + t
    x8_d = nc.dram_tensor("x8", [2, 128, NS, 2, TP], f8, kind="ExternalInput")
    # wth pre-tiled on host into chunk-major layout: [8, 128, FC, E]
    wth_d = nc.dram_tensor("wth", [8, 128, F // 8, E], f16,
                           kind="ExternalInput")
    # fp8 concat residual, stationary operand: [k, s, j, e]
    w8_d = nc.dram_tensor("w8", [128, NS, 2, E], f8, kind="ExternalInput")
    eye_d = nc.dram_tensor("eye", [128, 128], f16, kind="ExternalInput")
    nbias_d = nc.dram_tensor("nbias", [128, E], f32, kind="ExternalInput")
    iota_d = nc.dram_tensor("iota", [128, E], f32, kind="ExternalInput")
    out_d = nc.dram_tensor("out", [128, nt * 2 * K], u32, kind="ExternalOutput")

    with tile.TileContext(nc) as tc:
        with (
            tc.tile_pool(name="const", bufs=1) as cpool,
            tc.tile_pool(name="xp", bufs=6) as xpool,
            tc.tile_pool(name="x8p", bufs=1) as x8pool,
            tc.tile_pool(name="rs", bufs=2) as rspool,
            tc.tile_pool(name="sc", bufs=4) as spool,
            tc.tile_pool(name="sm", bufs=4) as smpool,
            tc.tile_pool(name="acc", bufs=4, space="PSUM") as apool,
            tc.tile_pool(name="rp", bufs=1, space="PSUM") as rppool,
        ):
            # sync ring: wth chunks 0-1 first (main tile 0 needs chunk 0
            # asap; xh t0 is interleaved next, then the rest).
            FC = F // 8
            wth_c = []
            for c in range(8):
                th = cpool.tile([128, FC, E], f16, tag=f"wth{c}",
                                name=f"wth{c}")
                if c < 2:
                    nc.sync.dma_start(th[:], wth_d.ap()[c])
                wth_c.append(th)
            # scalar ring: w8 halves, then x8 passes stream in the loop below
            w8_c = []
            for c in range(2):
                ss = slice(c * (NS // 2), (c + 1) * (NS // 2))
                t8 = cpool.tile([128, NS // 2, 2, E], f8, tag=f"w8{c}",
                                name=f"w8{c}")
                nc.gpsimd.dma_start(t8[:], w8_d.ap()[:, ss, :, :])
                w8_c.append(t8)
            eye = cpool.tile([128, 128], f16, tag="eye")
            nc.gpsimd.dma_start(eye[:], eye_d.ap())
            nb = cpool.tile([128, E], f32, tag="nb")
            nc.gpsimd.dma_start(nb[:], nbias_d.ap())
            io = cpool.tile([128, E], f32, tag="io")
            nc.gpsimd.dma_start(io[:], iota_d.ap())
            scrv = cpool.tile([128, E], f32, tag="scrv")
            scrg = cpool.tile([128, E], f32, tag="scrg")
            obuf = cpool.tile([128, nt * 2 * K], u32, tag="obuf")

            WC2 = NS // 2

            def main_mms(it, close_with_rs=None):
                """xh DMA + 32 main matmuls for token tile it. Returns acc.
                If close_with_rs=(rs, it4), also emits the two residual
                transpose-accumulate matmuls and closes the group."""
                xh = xpool.tile([128, F, TCHUNK], f16, tag="xh", name="xh")
                nc.sync.dma_start(
                    xh[:], xth_d.ap()[it].rearrange("p (f t) -> p f t", f=F))
                if it == 0:
                    for c in range(2, 8):
                        nc.sync.dma_start(wth_c[c][:], wth_d.ap()[c])
                acc = apool.tile([128, E], f32, tag="acc", name="acc")
                for f in range(F):
                    nc.tensor.matmul(acc[:], xh[:, f, :],
                                     wth_c[f // FC][:, f % FC, :],
                                     start=(f == 0), stop=False)
                if close_with_rs is not None:
                    tr_close(acc, *close_with_rs)
                return acc

            def tr_close(acc, rs, it4):
                for h in range(2):
                    nc.tensor.matmul(
                        acc[:, h * 128:(h + 1) * 128],
                        rs[h][:, it4 * 128:(it4 + 1) * 128],
                        eye[:], start=False, stop=(h == 1))

            def topk(acc, it):
                scores = spool.tile([128, E], f32, tag="scores", name="scores")
                nc.scalar.activation(scores[:], acc[:], Act.Sigmoid)

                negb = spool.tile([128, E], f32, tag="negb", name="negb")
                nc.vector.tensor_tensor(negb[:], nb[:], scores[:],
                                        Alu.subtract)
                m8 = smpool.tile([128, K], f32, tag="m8", name="m8")
                idx = obuf[:, it * 2 * K: it * 2 * K + K]
                nc.vector.max(m8[:], negb[:])
                nc.vector.max_index(idx, m8[:], negb[:])
                idxf = smpool.tile([128, K], f32, tag="idxf", name="idxf")
                nc.vector.tensor_copy(idxf[:], idx)

                # iota-equality gather of the selected scores (DVE)
                gath = smpool.tile([128, K], f32, tag="gath", name="gath")
                for j in range(K):
                    nc.vector.scalar_tensor_tensor(
                        scrv[:], io[:], idxf[:, j:j + 1], scores[:],
                        Alu.is_equal, Alu.mult,
                        accum_out=gath[:, j:j + 1])

                # sum(gath)/2.5 on the ACT engine, reciprocal on DVE,
                # final gath * (2.5/sum) back on ACT
                g8s = smpool.tile([128, K], f32, tag="g8s", name="g8s")
                ssum = smpool.tile([128, 1], f32, tag="ssum", name="ssum")
                nc.scalar.activation(g8s[:], gath[:], Act.Copy,
                                     scale=1.0 / ROUTED_SCALING,
                                     accum_out=ssum[:])
                rec = smpool.tile([128, 1], f32, tag="rec", name="rec")
                nc.vector.reciprocal(rec[:], ssum[:])
                nc.scalar.activation(
                    obuf[:, it * 2 * K + K: (it + 1) * 2 * K].bitcast(f32),
                    gath[:], Act.Copy, scale=rec[:])

            for p in range(2):
                # fp8 moving operand for this 1024-token pass (scalar ring,
                # 4 x 1 MiB chunk tiles released as the s-loop passes them)
                x8t = []
                for c in range(4):
                    t8 = x8pool.tile([128, NS // 4, 2, TP], f8,
                                     tag=f"x8{c}", name=f"x8{c}")
                    nc.gpsimd.dma_start(
                        t8[:], x8_d.ap()[p][:, c * (NS // 4):
                                            (c + 1) * (NS // 4), :, :])
                    x8t.append(t8)

                rp = [[rppool.tile([128, 512], f32, tag=f"rp{h}{c2}",
                                   name=f"rp{h}{c2}")
                       for c2 in range(2)] for h in range(2)]
                rs = [[rspool.tile([128, 512], f16, tag=f"rs{h}{c2}",
                                   name=f"rs{h}{c2}")
                       for c2 in range(2)] for h in range(2)]
                NSC = NS // 4

                def resid_mms(c2, s_range):
                    for s in s_range:
                        for h in range(2):
                            wsl = w8_c[s // WC2][:, s % WC2, :,
                                                 h * 128:(h + 1) * 128]
                            xsl = x8t[s // NSC][:, s % NSC, :,
                                                c2 * 512:(c2 + 1) * 512]
                            nc.tensor.matmul(rp[h][c2][:], wsl, xsl,
                                             start=(s == 0),
                                             stop=(s == NS - 1),
                                             perf_mode=DR)

                if p == 0:
                    # pass 0 is DMA-ramp-limited: keep the PE on residual
                    # work (scalar-ring data only) for ~28us while the sync
                    # ring streams wth+xh; the DVE backlog from the 8
                    # back-to-back closes drains during pass 1's residual.
                    resid_mms(0, range(NS))
                    for h in range(2):
                        nc.scalar.activation(rs[h][0][:], rp[h][0][:],
                                             Act.Copy)
                    resid_mms(1, range(NS))
                    for h in range(2):
                        nc.scalar.activation(rs[h][1][:], rp[h][1][:],
                                             Act.Copy)
                    for i in range(8):
                        c2 = i // 4
                        acc = main_mms(i, ([rs[0][c2], rs[1][c2]], i % 4))
                        topk(acc, i)
                else:
                    # steady state: close tiles 0-3 one by one with the
                    # second-half residual interleaved as PE filler so the
                    # closes stay spread out for the DVE.
                    resid_mms(0, range(NS))
                    for h in range(2):
                        nc.scalar.activation(rs[h][0][:], rp[h][0][:],
                                             Act.Copy)
                    for i in range(4):
                        it = p * 8 + i
                        acc = main_mms(it, ([rs[0][0], rs[1][0]], i))
                        topk(acc, it)
                        resid_mms(1, range(i * NSC, (i + 1) * NSC))
                    for h in range(2):
                        nc.scalar.activation(rs[h][1][:], rp[h][1][:],
                                             Act.Copy)
                    for i in range(4):
                        it = p * 8 + 4 + i
                        acc = main_mms(it, ([rs[0][1], rs[1][1]], i))
                        topk(acc, it)

                # stream out this pass's results while the next pass runs
                # (scalar ring: keeps the sync ring free for xh)
                sl = slice(p * 8 * 2 * K, (p + 1) * 8 * 2 * K)
                nc.gpsimd.dma_start(out_d.ap()[:, sl], obuf[:, sl])

    nc.compile()
    return nc


def host_prep(x, weight, e_score_correction_bias):
    """fp16 hi parts + fp8 concat residuals, pre-tiled per core."""
    x = np.asarray(x, dtype=np.float32)
    w = np.asarray(weight, dtype=np.float32)
    b = np.asarray(e_score_correction_bias, dtype=np.float32)
    S = np.float32(2.0 ** RSHIFT)

    xh = x.astype(np.float16)
    xl32 = x - xh.astype(np.float32)
    x8l = (xl32 * S).astype(E4NP)            # [T, H] term-1 x side
    x8h = xh.astype(E4NP)                    # [T, H] term-2 x side

    def pretile(a):  # [TS, H] -> [NT, 128, F*TCHUNK]; [it,p,f,t]=a[it*128+t,32p+f]
        a = a.reshape(NT, TCHUNK, 128, F).transpose(0, 2, 3, 1)
        return np.ascontiguousarray(a).reshape(NT, 128, F * TCHUNK)

    def pretile8(al, ah):  # two [TS, H] fp8 -> [2, 128, NS, 2, TP]
        # concat contraction g = s*256 + j*128 + k; g<H -> al, else ah
        cat = np.concatenate([al, ah], axis=1)       # [TS, S2]
        a = cat.reshape(2, TP, NS, 2, 128).transpose(0, 4, 2, 3, 1)
        return np.ascontiguousarray(a)

    wt = np.ascontiguousarray(w.T)     # [H, E]
    wth = wt.astype(np.float16)
    wl32 = wt - wth.astype(np.float32)
    w8h = wth.astype(E4NP)                   # term-1 w side
    w8l = (wl32 * S).astype(E4NP)            # term-2 w side
    wcat = np.concatenate([w8h, w8l], axis=0)        # [S2, E]
    w8 = np.ascontiguousarray(
        wcat.reshape(NS, 2, 128, E).transpose(2, 0, 1, 3))  # [128, NS, 2, E]

    # wth chunk-major: [8, 128, FC, E] with [c,p,fc,e] = wt[32p + c*FC + fc, e]
    FC = F // 8
    wtt = wth.reshape(128, F, E)             # [p, f, e] with h = 32p + f
    wthc = np.ascontiguousarray(
        wtt.reshape(128, 8, FC, E).transpose(1, 0, 2, 3))

    eye = (np.eye(128, dtype=np.float32) * (2.0 ** -RSHIFT)).astype(np.float16)
    nbias = np.ascontiguousarray(np.broadcast_to(-b, (128, E)))
    iota = np.ascontiguousarray(np.broadcast_to(
        np.arange(E, dtype=np.float32), (128, E)))

    in_maps = []
    for c in range(N_CORES):
        sl = slice(c * TS, (c + 1) * TS)
        in_maps.append({
            "xth": pretile(xh[sl]),
            "x8": pretile8(x8l[sl], x8h[sl]),
            "wth": wthc,
            "w8": w8,
            "eye": eye,
            "nbias": nbias,
            "iota": iota,
        })
    return in_maps


def unpack(out_cores):
    """list of [128, NT*16] u32 -> (inds int32 [T, 8], sel float32 [T, 8])."""
    inds = np.empty((T, K), dtype=np.int32)
    sel = np.empty((T, K), dtype=np.float32)
    for c, o in enumerate(out_cores):
        o = o.reshape(128, NT, 2 * K).transpose(1, 0, 2)  # [it, p, 16]
        o = np.ascontiguousarray(o).reshape(TS, 2 * K)
        inds[c * TS:(c + 1) * TS] = o[:, :K].astype(np.int32)
        sel[c * TS:(c + 1) * TS] = o[:, K:].view(np.float32)
    return inds, sel


_NC_CACHE = {}


def _get_nc():
    if "nc" not in _NC_CACHE:
        _NC_CACHE["nc"] = build_nc()
    return _NC_CACHE["nc"]


def kernel(x, weight, e_score_correction_bias, _trace=False):
    in_maps = host_prep(x, weight, e_score_correction_bias)
    nc = _get_nc()
    res = bass_utils.run_bass_kernel_spmd(
        nc, in_maps, list(range(N_CORES)), trace=_trace)
    inds, sel = unpack([res.results[c]["out"] for c in range(N_CORES)])
    if _trace:
        kernel.last_results = res
    return inds, sel


# revision 21
# speedup vs baseline: 1.1010x; 1.1010x over previous
"""MoE gate kernel for Trainium2 (8 NeuronCores, SPMD data-parallel over tokens).

reference:
    scores = sigmoid(x @ W.T)            # [T, E] fp32
    biased = scores + bias
    inds   = top_k(-biased, 8).indices   # 8 smallest biased, ascending biased
    sel    = scores[inds] / sum * 2.5

Numerics (validated host-side on the exact seeded inputs):
    logits = xh@wh  +  ([e4(xl*2^12); e4(xh)] @ [[e4(wh)]; [e4(wl*2^12)]]) / 2^12
  where xh/wh are the fp16 hi parts and xl/wl the fp32 residuals. The single
  fp8 matmul over the concatenated 8192-deep contraction captures both first-
  order correction terms; combined rel-err vs the fp32 reference ~7e-3.

Device strategy (per core, 2048 tokens):
  - main term: x-stationary fp16 matmuls, PSUM comes out [tokens, experts]
    (32 h-slices x 16 token tiles, N=256).
  - residual: w-stationary fp8 DoubleRow matmuls (256-deep contraction per
    instruction), output [experts, tokens] in PSUM. Two 1024-token passes so
    residual PSUM (4 banks) + open accs (4 banks) fit in 8 banks.
  - fold-in: residual PSUM -> ACT copy to SBUF fp16, then one tiny fp16
    matmul per (tile, expert-half) against diag(2^-12): transposes,
    descales and accumulates into the main acc bank in one PE op (the
    dot products have a single nonzero -> exact).
  - ACT sigmoid (fp32 for the top-8 compare path + fp16 copy for gather),
    DVE max/max_index for top-8, fp16 iota-equality gather of the selected
    original scores, reciprocal-normalize, scale by 2.5.
  - DMA: big transfers only (1-4 MiB), wth+xh on the sync HWDGE ring,
    w8+x8+consts on the scalar ring, deep buffering to avoid ring-head
    blocking.
  - Output: one [128, NT*16] u32 buffer; per token 8 idx words + 8 fp32-bit
    sel words; host un-permutes.
"""

import sys

sys.path.insert(0, "/opt/trn_rl_repo")

import numpy as np
import ml_dtypes

import concourse.bacc as bacc
import concourse.mybir as mybir
import concourse.tile as tile
from concourse import bass_utils

T, H, E, K = 16384, 4096, 256, 8
N_CORES = 8
TS = T // N_CORES          # tokens per core
TCHUNK = 128               # tokens per PE tile (PSUM partition dim)
NT = TS // TCHUNK          # token tiles per core
F = H // 128               # h-slices per partition block
S2 = H * 2                 # fp8 concat contraction depth (8192)
NS = S2 // 256             # DoubleRow slices (32)
TP = TS // 2               # tokens per residual pass (1024)
ROUTED_SCALING = 2.5
RSHIFT = 12                # residual scale = 2^12

f32 = mybir.dt.float32
f16 = mybir.dt.float16
f8 = mybir.dt.float8e4
u32 = mybir.dt.uint32
Alu = mybir.AluOpType
Act = mybir.ActivationFunctionType
DR = mybir.MatmulPerfMode.DoubleRow

E4NP = ml_dtypes.float8_e4m3


def build_nc(nt=NT):
    """Build the SPMD Bass program for one core handling nt*TCHUNK tokens."""
    nc = bacc.Bacc("TRN2", target_bir_lowering=False, debug=False,
                   num_devices=N_CORES)

    # x pre-tiled on host: [it, p, f*TCHUNK + t] = x[it*TCHUNK + t, 32p + f]
    xth_d = nc.dram_tensor("xth", [nt, 128, F * TCHUNK], f16,
                           kind="ExternalInput")
    # fp8 concat residual, moving operand: [pass, k, s, j, t]
    #   h' = s*256 + j*128 + k in [0, 8192); tok = pass*TP + t
    x8_d = nc.dram_tensor("x8", [2, 128, NS, 2, TP], f8, kind="ExternalInput")
    # wth pre-tiled on host into chunk-major layout: [8, 128, FC, E]
    wth_d = nc.dram_tensor("wth", [8, 128, F // 8, E], f16,
                           kind="ExternalInput")
    # fp8 concat residual, stationary operand: [k, s, j, e]
    w8_d = nc.dram_tensor("w8", [128, NS, 2, E], f8, kind="ExternalInput")
    eye_d = nc.dram_tensor("eye", [128, 128], f16, kind="ExternalInput")
    nbias_d = nc.dram_tensor("nbias", [128, E], f32, kind="ExternalInput")
    iota_d = nc.dram_tensor("iota", [128, E], f32, kind="ExternalInput")
    out_d = nc.dram_tensor("out", [128, nt * 2 * K], u32, kind="ExternalOutput")

    with tile.TileContext(nc) as tc:
        with (
            tc.tile_pool(name="const", bufs=1) as cpool,
            tc.tile_pool(name="xp", bufs=6) as xpool,
            tc.tile_pool(name="x8p", bufs=1) as x8pool,
            tc.tile_pool(name="rs", bufs=2) as rspool,
            tc.tile_pool(name="sc", bufs=4) as spool,
            tc.tile_pool(name="sm", bufs=4) as smpool,
            tc.tile_pool(name="acc", bufs=4, space="PSUM") as apool,
            tc.tile_pool(name="rp", bufs=1, space="PSUM") as rppool,
        ):
            # sync ring carries, in order: the prologue's fp8 operands
            # (w8 + x8 pass 0), then wth, then the xh tile stream. The sync
            # engine runs no compute, so trigger-blocking is harmless.
            w8_c = []
            for c in range(2):
                ss = slice(c * (NS // 2), (c + 1) * (NS // 2))
                t8 = cpool.tile([128, NS // 2, 2, E], f8, tag=f"w8{c}",
                                name=f"w8{c}")
                nc.sync.dma_start(t8[:], w8_d.ap()[:, ss, :, :])
                w8_c.append(t8)

            def x8_dma(p):
                eng = nc.sync if p == 0 else nc.gpsimd
                x8t = []
                for c in range(4):
                    t8 = x8pool.tile([128, NS // 4, 2, TP], f8,
                                     tag=f"x8{c}", name=f"x8{c}")
                    eng.dma_start(
                        t8[:], x8_d.ap()[p][:, c * (NS // 4):
                                            (c + 1) * (NS // 4), :, :])
                    x8t.append(t8)
                return x8t

            x8t = [None, None]
            x8t[0] = x8_dma(0)

            FC = F // 8
            wth_c = []
            for c in range(8):
                th = cpool.tile([128, FC, E], f16, tag=f"wth{c}",
                                name=f"wth{c}")
                nc.sync.dma_start(th[:], wth_d.ap()[c])
                wth_c.append(th)

            # small consts on the gpsimd SWDGE path
            eye = cpool.tile([128, 128], f16, tag="eye")
            nc.gpsimd.dma_start(eye[:], eye_d.ap())
            nb = cpool.tile([128, E], f32, tag="nb")
            nc.gpsimd.dma_start(nb[:], nbias_d.ap())
            io = cpool.tile([128, E], f32, tag="io")
            nc.gpsimd.dma_start(io[:], iota_d.ap())
            scrv = cpool.tile([128, E], f32, tag="scrv")
            obuf = cpool.tile([128, nt * 2 * K], u32, tag="obuf")

            WC2 = NS // 2
            NSC = NS // 4

            rs_all = {}

            def resid_mms(p, c2, s_range):
                for s in s_range:
                    for h in range(2):
                        wsl = w8_c[s // WC2][:, s % WC2, :,
                                             h * 128:(h + 1) * 128]
                        xsl = x8t[p][s // NSC][:, s % NSC, :,
                                               c2 * 512:(c2 + 1) * 512]
                        nc.tensor.matmul(rp[h][c2][:], wsl, xsl,
                                         start=(s == 0),
                                         stop=(s == NS - 1),
                                         perf_mode=DR)

            def resid_copy(p, c2):
                rr = []
                for h in range(2):
                    t = rspool.tile([128, 512], f16, tag=f"rs{h}{c2}",
                                    name=f"rs{h}{c2}")
                    nc.scalar.activation(t[:], rp[h][c2][:], Act.Copy)
                    rr.append(t)
                rs_all[(p, c2)] = rr

            def tile_grp(it):
                """One token tile: tr-first accumulation group (the
                transposed-residual matmuls OPEN the group, so has_written
                ordering forces the mains to wait for the residual), then
                the 32 main matmuls, close on the last."""
                xh = xpool.tile([128, F, TCHUNK], f16, tag="xh", name="xh")
                nc.sync.dma_start(
                    xh[:], xth_d.ap()[it].rearrange("p (f t) -> p f t", f=F))
                acc = apool.tile([128, E], f32, tag="acc", name="acc")
                p, c2, it4 = it // 8, (it % 8) // 4, it % 4
                rr = rs_all[(p, c2)]
                for h in range(2):
                    nc.tensor.matmul(
                        acc[:, h * 128:(h + 1) * 128],
                        rr[h][:, it4 * 128:(it4 + 1) * 128],
                        eye[:], start=(h == 0), stop=False)
                for f in range(F):
                    nc.tensor.matmul(acc[:], xh[:, f, :],
                                     wth_c[f // FC][:, f % FC, :],
                                     start=False, stop=(f == F - 1))
                topk(acc, it)

            def topk(acc, it):
                scores = spool.tile([128, E], f32, tag="scores",
                                    name="scores")
                nc.scalar.activation(scores[:], acc[:], Act.Sigmoid)

                negb = spool.tile([128, E], f32, tag="negb", name="negb")
                nc.vector.tensor_tensor(negb[:], nb[:], scores[:],
                                        Alu.subtract)
                m8 = smpool.tile([128, K], f32, tag="m8", name="m8")
                idx = obuf[:, it * 2 * K: it * 2 * K + K]
                nc.vector.max(m8[:], negb[:])
                nc.vector.max_index(idx, m8[:], negb[:])
                idxf = smpool.tile([128, K], f32, tag="idxf", name="idxf")
                nc.vector.tensor_copy(idxf[:], idx)

                gath = smpool.tile([128, K], f32, tag="gath", name="gath")
                for j in range(K):
                    nc.vector.scalar_tensor_tensor(
                        scrv[:], io[:], idxf[:, j:j + 1], scores[:],
                        Alu.is_equal, Alu.mult,
                        accum_out=gath[:, j:j + 1])

                g8s = smpool.tile([128, K], f32, tag="g8s", name="g8s")
                ssum = smpool.tile([128, 1], f32, tag="ssum", name="ssum")
                nc.scalar.activation(g8s[:], gath[:], Act.Copy,
                                     scale=1.0 / ROUTED_SCALING,
                                     accum_out=ssum[:])
                rec = smpool.tile([128, 1], f32, tag="rec", name="rec")
                nc.vector.reciprocal(rec[:], ssum[:])
                nc.scalar.activation(
                    obuf[:, it * 2 * K + K: (it + 1) * 2 * K].bitcast(f32),
                    gath[:], Act.Copy, scale=rec[:])

            # prologue: residual for pass 0 runs alone while wth + the
            # first xh tiles stream on the sync ring
            rp = [[rppool.tile([128, 512], f32, tag=f"rp{h}{c2}",
                               name=f"rp{h}{c2}")
                   for c2 in range(2)] for h in range(2)]
            for c2 in range(2):
                resid_mms(0, c2, range(NS))
                resid_copy(0, c2)

            # pass 0: tiles 0-7, with pass 1's residual interleaved as PE
            # filler (quarter per tile, c2=0 behind tiles 0-3, c2=1 behind
            # tiles 4-7)
            x8t[1] = x8_dma(1)
            rp = [[rppool.tile([128, 512], f32, tag=f"rp{h}{c2}",
                               name=f"rp{h}{c2}")
                   for c2 in range(2)] for h in range(2)]
            for i in range(8):
                tile_grp(i)
                c2, q = i // 4, i % 4
                resid_mms(1, c2, range(q * NSC, (q + 1) * NSC))
                if i == 3 or i == 7:
                    resid_copy(1, c2)
            nc.gpsimd.dma_start(out_d.ap()[:, :8 * 2 * K],
                                obuf[:, :8 * 2 * K])

            # pass 1: tiles 8-15 (their residual is already in SBUF)
            for i in range(8, 16):
                tile_grp(i)
            nc.gpsimd.dma_start(out_d.ap()[:, 8 * 2 * K:],
                                obuf[:, 8 * 2 * K:])

    nc.compile()
    return nc


def host_prep(x, weight, e_score_correction_bias):
    """fp16 hi parts + fp8 concat residuals, pre-tiled per core."""
    x = np.asarray(x, dtype=np.float32)
    w = np.asarray(weight, dtype=np.float32)
    b = np.asarray(e_score_correction_bias, dtype=np.float32)
    S = np.float32(2.0 ** RSHIFT)

    xh = x.astype(np.float16)
    xl32 = x - xh.astype(np.float32)
    x8l = (xl32 * S).astype(E4NP)            # [T, H] term-1 x side
    x8h = xh.astype(E4NP)                    # [T, H] term-2 x side

    def pretile(a):  # [TS, H] -> [NT, 128, F*TCHUNK]; [it,p,f,t]=a[it*128+t,32p+f]
        a = a.reshape(NT, TCHUNK, 128, F).transpose(0, 2, 3, 1)
        return np.ascontiguousarray(a).reshape(NT, 128, F * TCHUNK)

    def pretile8(al, ah):  # two [TS, H] fp8 -> [2, 128, NS, 2, TP]
        # concat contraction g = s*256 + j*128 + k; g<H -> al, else ah
        cat = np.concatenate([al, ah], axis=1)       # [TS, S2]
        a = cat.reshape(2, TP, NS, 2, 128).transpose(0, 4, 2, 3, 1)
        return np.ascontiguousarray(a)

    wt = np.ascontiguousarray(w.T)     # [H, E]
    wth = wt.astype(np.float16)
    wl32 = wt - wth.astype(np.float32)
    w8h = wth.astype(E4NP)                   # term-1 w side
    w8l = (wl32 * S).astype(E4NP)            # term-2 w side
    wcat = np.concatenate([w8h, w8l], axis=0)        # [S2, E]
    w8 = np.ascontiguousarray(
        wcat.reshape(NS, 2, 128, E).transpose(2, 0, 1, 3))  # [128, NS, 2, E]

    # wth chunk-major: [8, 128, FC, E] with [c,p,fc,e] = wt[32p + c*FC + fc, e]
    FC = F // 8
    wtt = wth.reshape(128, F, E)             # [p, f, e] with h = 32p + f
    wthc = np.ascontiguousarray(
        wtt.reshape(128, 8, FC, E).transpose(1, 0, 2, 3))

    eye = (np.eye(128, dtype=np.float32) * (2.0 ** -RSHIFT)).astype(np.float16)
    nbias = np.ascontiguousarray(np.broadcast_to(-b, (128, E)))
    iota = np.ascontiguousarray(np.broadcast_to(
        np.arange(E, dtype=np.float32), (128, E)))

    in_maps = []
    for c in range(N_CORES):
        sl = slice(c * TS, (c + 1) * TS)
        in_maps.append({
            "xth": pretile(xh[sl]),
            "x8": pretile8(x8l[sl], x8h[sl]),
            "wth": wthc,
            "w8": w8,
            "eye": eye,
            "nbias": nbias,
            "iota": iota,
        })
    return in_maps


def unpack(out_cores):
    """list of [128, NT*16] u32 -> (inds int32 [T, 8], sel float32 [T, 8])."""
    inds = np.empty((T, K), dtype=np.int32)
    sel = np.empty((T, K), dtype=np.float32)
    for c, o in enumerate(out_cores):
        o = o.reshape(128, NT, 2 * K).transpose(1, 0, 2)  # [it, p, 16]
        o = np.ascontiguousarray(o).reshape(TS, 2 * K)
        inds[c * TS:(c + 1) * TS] = o[:, :K].astype(np.int32)
        sel[c * TS:(c + 1) * TS] = o[:, K:].view(np.float32)
    return inds, sel


_NC_CACHE = {}


def _get_nc():
    if "nc" not in _NC_CACHE:
        _NC_CACHE["nc"] = build_nc()
    return _NC_CACHE["nc"]


def kernel(x, weight, e_score_correction_bias, _trace=False):
    in_maps = host_prep(x, weight, e_score_correction_bias)
    nc = _get_nc()
    res = bass_utils.run_bass_kernel_spmd(
        nc, in_maps, list(range(N_CORES)), trace=_trace)
    inds, sel = unpack([res.results[c]["out"] for c in range(N_CORES)])
    if _trace:
        kernel.last_results = res
    return inds, sel
